# revision 21
# baseline (speedup 1.0000x reference)
"""EyesMouthLoss Trainium2 kernel.

loss = mean(|pred-target| * (1 + 299*clip(eye_mask+mouth_mask, 0, 1)))

Sharding: pure data-parallel over B=16 -> 2 batches per core on 8 cores.
Host sums the 8 per-core partial scalars (the final all-reduce).

Key idea: region = relu(1 - dist/15) is zero beyond 14px, and the mask
around every landmark is the SAME constant 29x29 radial stencil, just
translated.  So each (eye/mouth) field is built by max-ing the stencil
into a zeroed field at ~32 windows per batch (tiny DVE ops), instead of
32 full-image passes.  Landmark coordinates are known at compile time
(the program is specialized to the inputs); per-core divergence of the
window addresses is handled with a tc.Switch on the partition id.

Compute-engine APs must start at a partition multiple of 32, so each
window op is padded down to the 32-aligned partition start, and the
stencil operand comes from a bank of 60 row-shifted stencil images
RP[64, 60, 29] (RP[r, t, j] = region(r-(t-14), j-14)); rows outside
the real window read stencil values beyond radius 15, which are 0 and
are identity under max with a non-negative field.
"""

import sys

sys.path.insert(0, "/opt/trn_rl_repo")

from contextlib import ExitStack

import numpy as np

import concourse.bass as bass
import concourse.tile as tile
from concourse import bacc
from concourse import mybir
from concourse.bass_utils import run_bass_kernel_spmd

B, C, H, W = 16, 3, 512, 512
NCORES = 8
BPC = B // NCORES  # batches per core
RADIUS = 15.0
HALF = 14  # region strictly zero for |dx| >= 15
WIN = 2 * HALF + 1  # 29
NSHIFT = 156  # row shifts: t = cy-128k+14 in [0, 155]
EYE = (36, 48)
MOUTH = (48, 68)
WEIGHT = 300.0
NTOT = float(B * C * H * W)
FP32 = mybir.dt.float32
I32 = mybir.dt.int32
Alu = mybir.AluOpType
Act = mybir.ActivationFunctionType


def _windows_for(lm_b, lo, hi):
    """Window pieces (t, k, x0, ncols, sc0) for one landmark group.

    Field layout [128 partitions = y%128, 4 chunks = y//128, 512 x].
    Each piece is one full-height DVE max op on chunk k, cols
    [x0, x0+ncols), with stencil operand RP[:, t, sc0:sc0+ncols].
    Partitions outside the real window read stencil values beyond
    radius 15 (= 0), which are identity under max with fields >= 0;
    compute APs may only start at partition 0 on this toolchain unless
    32-aligned/32-wide, so full height is both legal and free (DVE cost
    scales with the free dim only).
    """
    pieces = []
    seen = set()
    for cx, cy in lm_b[lo:hi]:
        cx = int(min(max(int(cx), 0), W - 1))
        cy = int(min(max(int(cy), 0), H - 1))
        if (cx, cy) in seen:
            continue
        seen.add((cx, cy))
        y0, y1 = max(0, cy - HALF), min(H - 1, cy + HALF)
        x0, x1 = max(0, cx - HALF), min(W - 1, cx + HALF)
        sc0 = x0 - (cx - HALF)
        ncols = x1 - x0 + 1
        for k in range(y0 >> 7, (y1 >> 7) + 1):
            t = cy - 128 * k + 14
            assert 0 <= t < NSHIFT
            pieces.append((t, k, x0, ncols, sc0))
    return pieces


def _build(landmarks):
    """Build the SPMD Bass program, specialized to the landmark values."""
    nc = bacc.Bacc(None)
    pred_p = nc.declare_dram_parameter("pred", [BPC, C, H, W], FP32, isOutput=False)
    targ_p = nc.declare_dram_parameter("targ", [BPC, C, H, W], FP32, isOutput=False)
    out_p = nc.declare_dram_parameter("out", [1, 1], FP32, isOutput=True)

    with tile.TileContext(nc) as tc, ExitStack() as ctx:
        stat_pool = ctx.enter_context(tc.tile_pool(name="stat", bufs=4))
        const_pool = ctx.enter_context(tc.tile_pool(name="const", bufs=1))
        psum_pool = ctx.enter_context(tc.tile_pool(name="psum", bufs=1, space="PSUM"))

        # ---- constants ----
        w_base = const_pool.tile([128, 1], FP32)  # 1/N
        nc.gpsimd.memset(w_base[:], 1.0 / NTOT)
        w_prio = const_pool.tile([128, 1], FP32)  # (WEIGHT-1)/N
        nc.gpsimd.memset(w_prio[:], (WEIGHT - 1.0) / NTOT)

        # ---- shifted radial stencil bank ----
        # rp[p, t, j] = relu(1 - sqrt((p-(t-14))^2 + (j-14)^2)/15)
        # (scratch tiles stay allocated: releasing them would recycle their
        # SBUF for the streaming tiles and attach multi-sem WAR waits to the
        # load DMAs, which this toolchain's DMA encoding cannot carry)
        rp = const_pool.tile([128, NSHIFT, WIN], FP32)
        f_r = const_pool.tile([128, NSHIFT, WIN], FP32)
        nc.gpsimd.iota(f_r[:], pattern=[[-1, NSHIFT], [0, WIN]], base=14,
                       channel_multiplier=1,
                       allow_small_or_imprecise_dtypes=True)
        f_c = const_pool.tile([128, NSHIFT, WIN], FP32)
        nc.gpsimd.iota(f_c[:], pattern=[[0, NSHIFT], [1, WIN]], base=-HALF,
                       channel_multiplier=0,
                       allow_small_or_imprecise_dtypes=True)
        nc.vector.tensor_tensor(f_r[:], f_r[:], f_r[:], op=Alu.mult)
        nc.vector.tensor_tensor(f_c[:], f_c[:], f_c[:], op=Alu.mult)
        nc.vector.tensor_tensor(f_r[:], f_r[:], f_c[:], op=Alu.add)
        nc.scalar.activation(f_c[:], f_r[:], Act.Sqrt)
        nc.scalar.activation(rp[:], f_c[:], Act.Relu, bias=1.0,
                             scale=-1.0 / RADIUS)

        load_pool = ctx.enter_context(tc.tile_pool(name="load", bufs=2))
        field_pool = ctx.enter_context(tc.tile_pool(name="field", bufs=2))

        psum = psum_pool.tile([1, 1], FP32)
        psum_scr = psum_pool.tile([1, 1], FP32)
        core_idx = nc.vector.partition_id()

        # The TT/ACT/etc ISA structs on this toolchain accept at most ONE
        # sync-wait per instruction.  "Joiner" micro-copies make each engine
        # observe each foreign semaphore once (1 wait each); heavy ops then
        # depend on joiners of the same engine (in-order, no extra sem).
        from concourse.tile import add_dep_helper

        def dep(after, *joiners):
            for j in joiners:
                add_dep_helper(after.ins, j.ins, reason="single-wait joiner order")

        _jn = [0]

        def jtile():
            _jn[0] += 1
            return const_pool.tile([1, 2], FP32, name=f"jt{_jn[0]}", tag=f"jt{_jn[0]}")

        # one-time joiners: DVE observes ACT's stencil tick, PE observes Pool
        jd_rp = nc.vector.tensor_copy(jtile()[:], rp[0:1, 0, 0:2])
        pe_j0 = nc.tensor.matmul(
            psum_scr[:], w_base[:], w_prio[:], start=True, stop=True
        )

        n_mm = 0
        total_mms = 2 * BPC

        for bi in range(BPC):
            # ---- stream pred/target, layout [p=y%128, c, k=y//128, x] ----
            p_t = load_pool.tile([128, C, 4, W], FP32, tag="p_t")
            t_t = load_pool.tile([128, C, 4, W], FP32, tag="t_t")
            nc.sync.dma_start(
                p_t[:], pred_p[bi].rearrange("c (k p) x -> p c k x", p=128)
            )
            nc.sync.dma_start(
                t_t[:], targ_p[bi].rearrange("c (k p) x -> p c k x", p=128)
            )

            # ---- priority fields ----
            e_f = field_pool.tile([128, 4, W], FP32, tag="e_f")
            m_f = field_pool.tile([128, 4, W], FP32, tag="m_f")
            nc.gpsimd.memset(e_f[:], 0.0)
            nc.gpsimd.memset(m_f[:], 0.0)

            # joiners (see above): one foreign sem each
            jd_e_t = jtile()
            jd_e = nc.vector.tensor_copy(jd_e_t[:], e_f[0:1, 0, 0:2])
            jd_m = nc.vector.tensor_copy(jtile()[:], m_f[0:1, 0, 0:2])
            jd_p = nc.vector.tensor_copy(jtile()[:], p_t[0:1, 0, 0, 0:2])
            jd_t = nc.vector.tensor_copy(jtile()[:], t_t[0:1, 0, 0, 0:2])
            ja_p = nc.scalar.copy(jtile()[:], p_t[0:1, 0, 0, 0:2])
            ja_t = nc.scalar.copy(jtile()[:], t_t[0:1, 0, 0, 0:2])
            jp_t = nc.gpsimd.tensor_copy(jtile()[:], t_t[0:1, 0, 0, 0:2])

            # per-core landmark windows (compile-time addresses, runtime dispatch)
            for case in range(NCORES):
                lm_b = landmarks[case * BPC + bi]
                with tc.If(core_idx == case):
                    for field, lo, hi in (
                        (e_f, EYE[0], EYE[1]),
                        (m_f, MOUTH[0], MOUTH[1]),
                    ):
                        for t, k, x0, ncols, sc0 in _windows_for(lm_b, lo, hi):
                            w = nc.vector.tensor_tensor(
                                field[:, k, x0 : x0 + ncols],
                                field[:, k, x0 : x0 + ncols],
                                rp[:, t, sc0 : sc0 + ncols],
                                op=Alu.max,
                            )
                            dep(w, jd_rp, jd_e, jd_m)
                            last_w = w

            # absorb the If-reconverge DVE self-wait so the sub below can
            # carry its single allowed wait on a foreign semaphore
            jd_bal = nc.vector.tensor_copy(jtile()[:], jd_e_t[0:1, 0:2])
            dep(jd_bal, last_w)
            # ---- d = pred - target (in place into p_t) ----
            sub = nc.vector.tensor_tensor(p_t[:], p_t[:], t_t[:], op=Alu.subtract)
            dep(sub, jd_p, jd_t, jd_bal)
            # ---- |d| into t_t; accum_out = per-partition sum of all |d| ----
            rs_s = stat_pool.tile([128, 1], FP32, tag="rs_s")
            ab = nc.scalar.activation(t_t[:], p_t[:], Act.Abs, accum_out=rs_s[:])
            dep(ab, ja_p, ja_t)
            # ---- S = sum over channels, into t_t[:,0] (gpsimd offload) ----
            cs1 = nc.gpsimd.tensor_tensor(t_t[:, 0], t_t[:, 0], t_t[:, 1], op=Alu.add)
            dep(cs1, jp_t)
            nc.gpsimd.tensor_tensor(t_t[:, 0], t_t[:, 0], t_t[:, 2], op=Alu.add)
            # DVE re-observes ACT (abs) before reading its outputs
            jd_a = nc.vector.tensor_copy(jtile()[0:1, 0:1], rs_s[0:1, 0:1])
            # ---- t = e + m (into e_f), p = min(t,1) (into m_f) ----
            nc.vector.tensor_tensor(e_f[:], e_f[:], m_f[:], op=Alu.add)
            nc.vector.tensor_scalar(m_f[:], e_f[:], 1.0, None, op0=Alu.min)
            # ---- f = p * S (into m_f), then row-sum ----
            ml = nc.vector.tensor_tensor(m_f[:], m_f[:], t_t[:, 0], op=Alu.mult)
            dep(ml, jd_a)
            rs_f = stat_pool.tile([128, 1], FP32, tag="rs_f")
            nc.vector.tensor_reduce(
                rs_f[:], m_f[:], axis=mybir.AxisListType.XY, op=Alu.add
            )
            # ---- accumulate into PSUM: (1/N)*sum(S) + (299/N)*sum(S*p) ----
            mm1 = nc.tensor.matmul(
                psum[:], w_base[:], rs_s[:], start=(n_mm == 0), stop=False
            )
            n_mm += 1
            mm2 = nc.tensor.matmul(
                psum[:], w_prio[:], rs_f[:], start=False, stop=(n_mm == total_mms - 1)
            )
            n_mm += 1
            dep(mm1, pe_j0)
            dep(mm2, pe_j0)

        res = const_pool.tile([1, 1], FP32)
        nc.scalar.copy(res[:], psum[:])
        nc.sync.dma_start(out_p[:], res[:])

    return nc


def run(inputs, trace=False):
    pred = np.ascontiguousarray(inputs["pred"], dtype=np.float32)
    targ = np.ascontiguousarray(inputs["target"], dtype=np.float32)
    lms = np.asarray(inputs["landmarks"])
    assert pred.shape == (B, C, H, W) and targ.shape == (B, C, H, W)

    nc = _build(lms)
    nc.finalize()
    in_maps = [
        {
            "pred": pred[i * BPC : (i + 1) * BPC],
            "targ": targ[i * BPC : (i + 1) * BPC],
        }
        for i in range(NCORES)
    ]
    res = run_bass_kernel_spmd(nc, in_maps, list(range(NCORES)), trace=trace)
    total = np.float32(0.0)
    for i in range(NCORES):
        total += np.float32(res.results[i]["out"][0, 0])
    return np.float32(total), res


def kernel(pred, target, landmarks):
    out, _ = run({"pred": pred, "target": target, "landmarks": landmarks})
    return out


# revision 22
# speedup vs baseline: 1.0887x; 1.0887x over previous
"""EyesMouthLoss Trainium2 kernel.

loss = mean(|pred-target| * (1 + 299*clip(eye_mask+mouth_mask, 0, 1)))

Sharding: pure data-parallel over B=16 -> 2 batches per core on 8 cores.
Host sums the 8 per-core partial scalars (the final all-reduce).

Key idea: region = relu(1 - dist/15) is zero beyond 14px, and the mask
around every landmark is the SAME constant 29x29 radial stencil, just
translated.  So each (eye/mouth) field is built by max-ing a stencil
into a zeroed field at ~32 windows per batch (tiny DVE ops), instead of
32 full-image passes.  Landmark coordinates are known at compile time
(the program is specialized to the inputs); per-core divergence of the
window addresses is a tc.Switch on the partition id.

Compute APs must start at partition 0 (or 32-aligned <=32-wide) on this
toolchain, so every window op spans all 128 partitions; rows outside
the real window read stencil values beyond radius 15, which are 0 and
identity under max with the non-negative field.  The stencil operand
comes from a bank of 156 row-shifted stencil images
rp[p, t, j] = region(p - (t-14), j - 14), t = cy - 128*chunk + 14.

The weighted mean uses min(t,1) = 1 - relu(1-t):
  sum S*(1+299*min(e+m,1)) = 300*sum(S) - 299*sum(S*relu(1-e-m)).
"""

import sys

sys.path.insert(0, "/opt/trn_rl_repo")

from contextlib import ExitStack

import numpy as np

import concourse.bass as bass
import concourse.tile as tile
from concourse import bacc, mybir
from concourse.bass_utils import run_bass_kernel_spmd

B, C, H, W = 16, 3, 512, 512
NCORES = 8
BPC = B // NCORES  # batches per core
RADIUS = 15.0
HALF = 14  # region strictly zero for |dx| >= 15
WIN = 2 * HALF + 1  # 29
NSHIFT = 156  # row shifts: t = cy-128k+14 in [0, 155]
EYE = (36, 48)
MOUTH = (48, 68)
WEIGHT = 300.0
NTOT = float(B * C * H * W)
FP32 = mybir.dt.float32
Alu = mybir.AluOpType
Act = mybir.ActivationFunctionType


def _windows_for(lm_b, lo, hi):
    """Window pieces (t, k, x0, ncols, sc0) for one landmark group."""
    pieces = []
    seen = set()
    for cx, cy in lm_b[lo:hi]:
        cx = int(min(max(int(cx), 0), W - 1))
        cy = int(min(max(int(cy), 0), H - 1))
        if (cx, cy) in seen:
            continue
        seen.add((cx, cy))
        y0, y1 = max(0, cy - HALF), min(H - 1, cy + HALF)
        x0, x1 = max(0, cx - HALF), min(W - 1, cx + HALF)
        sc0 = x0 - (cx - HALF)
        ncols = x1 - x0 + 1
        for k in range(y0 >> 7, (y1 >> 7) + 1):
            t = cy - 128 * k + 14
            assert 0 <= t < NSHIFT
            pieces.append((t, k, x0, ncols, sc0))
    return pieces


def _build(landmarks):
    """Build the SPMD Bass program, specialized to the landmark values."""
    nc = bacc.Bacc(None)
    pred_p = nc.declare_dram_parameter("pred", [BPC, C, H, W], FP32, isOutput=False)
    targ_p = nc.declare_dram_parameter("targ", [BPC, C, H, W], FP32, isOutput=False)
    out_p = nc.declare_dram_parameter("out", [1, 1], FP32, isOutput=True)

    with tile.TileContext(nc) as tc, ExitStack() as ctx:
        stat_pool = ctx.enter_context(tc.tile_pool(name="stat", bufs=4))
        const_pool = ctx.enter_context(tc.tile_pool(name="const", bufs=1))
        psum_pool = ctx.enter_context(tc.tile_pool(name="psum", bufs=1, space="PSUM"))

        # ---- constants ----
        w_s = const_pool.tile([128, 1], FP32)  # 300/N
        nc.gpsimd.memset(w_s[:], WEIGHT / NTOT)
        w_g = const_pool.tile([128, 1], FP32)  # -299/N
        nc.gpsimd.memset(w_g[:], -(WEIGHT - 1.0) / NTOT)

        # ---- shifted radial stencil bank (separable, via broadcast APs) ----
        # rp[p, t, j] = relu(1 - sqrt((p-t+14)^2 + (j-14)^2)/15)
        rowv = const_pool.tile([128, NSHIFT], FP32)
        nc.gpsimd.iota(rowv[:], pattern=[[-1, NSHIFT]], base=14,
                       channel_multiplier=1, allow_small_or_imprecise_dtypes=True)
        colv = const_pool.tile([128, WIN], FP32)
        nc.gpsimd.iota(colv[:], pattern=[[1, WIN]], base=-HALF,
                       channel_multiplier=0, allow_small_or_imprecise_dtypes=True)
        nc.vector.tensor_tensor(rowv[:], rowv[:], rowv[:], op=Alu.mult)
        nc.vector.tensor_tensor(colv[:], colv[:], colv[:], op=Alu.mult)
        bank_a = const_pool.tile([128, NSHIFT, WIN], FP32)
        bank_b = const_pool.tile([128, NSHIFT, WIN], FP32)
        nc.vector.tensor_tensor(
            bank_a[:],
            rowv[:].broadcast_to([128, NSHIFT, WIN]),
            colv[:].broadcast_to([128, WIN, NSHIFT]).rearrange("p j t -> p t j"),
            op=Alu.add,
        )
        nc.scalar.activation(bank_b[:], bank_a[:], Act.Sqrt)
        rp = bank_a
        nc.scalar.activation(rp[:], bank_b[:], Act.Relu, bias=1.0,
                             scale=-1.0 / RADIUS)

        load_pool = ctx.enter_context(tc.tile_pool(name="load", bufs=2))
        field_pool = ctx.enter_context(tc.tile_pool(name="field", bufs=2))

        psum = psum_pool.tile([1, 512], FP32)
        core_idx = nc.vector.partition_id()

        n_mm = 0
        total_mms = BPC * 5

        for bi in range(BPC):
            # ---- stream pred/target, layout [p=y%128, c, k=y//128, x] ----
            p_t = load_pool.tile([128, C, 4, W], FP32, tag="p_t")
            t_t = load_pool.tile([128, C, 4, W], FP32, tag="t_t")
            nc.sync.dma_start(
                p_t[:], pred_p[bi].rearrange("c (k p) x -> p c k x", p=128)
            )
            nc.sync.dma_start(
                t_t[:], targ_p[bi].rearrange("c (k p) x -> p c k x", p=128)
            )

            # ---- priority fields ----
            e_f = field_pool.tile([128, 4, W], FP32, tag="e_f")
            m_f = field_pool.tile([128, 4, W], FP32, tag="m_f")
            nc.gpsimd.memset(e_f[:], 0.0)
            nc.gpsimd.memset(m_f[:], 0.0)

            # per-core landmark windows (compile-time addresses, runtime dispatch)
            for case in tc.Switch(core_idx, NCORES):
                lm_b = landmarks[case * BPC + bi]
                for field, lo, hi in (
                    (e_f, EYE[0], EYE[1]),
                    (m_f, MOUTH[0], MOUTH[1]),
                ):
                    for t, k, x0, ncols, sc0 in _windows_for(lm_b, lo, hi):
                        nc.vector.tensor_tensor(
                            field[:, k, x0 : x0 + ncols],
                            field[:, k, x0 : x0 + ncols],
                            rp[:, t, sc0 : sc0 + ncols],
                            op=Alu.max,
                        )

            # ---- d = pred - target (in place into p_t) ----
            nc.vector.tensor_tensor(p_t[:], p_t[:], t_t[:], op=Alu.subtract)
            # ---- |d| into t_t; accum_out = per-partition sum of all |d| ----
            rs_s = stat_pool.tile([128, 1], FP32, tag="rs_s")
            nc.scalar.activation(t_t[:], p_t[:], Act.Abs, accum_out=rs_s[:])
            # ---- S = sum over channels into t_t[:,0] (split Pool/DVE) ----
            nc.gpsimd.tensor_tensor(t_t[:, 0], t_t[:, 0], t_t[:, 1], op=Alu.add)
            nc.vector.tensor_tensor(t_t[:, 0], t_t[:, 0], t_t[:, 2], op=Alu.add)
            # ---- t = e + m (Pool), r = relu(1 - t) (ACT), g = S*r (DVE) ----
            nc.gpsimd.tensor_tensor(e_f[:], e_f[:], m_f[:], op=Alu.add)
            nc.scalar.activation(m_f[:], e_f[:], Act.Relu, bias=1.0, scale=-1.0)
            nc.vector.tensor_tensor(m_f[:], m_f[:], t_t[:, 0], op=Alu.mult)
            # ---- accumulate into PSUM: 300/N*sum(S) - 299/N*sum(S*r) ----
            nc.tensor.matmul(
                psum[0:1, 0:1], w_s[:], rs_s[:], start=(n_mm == 0), stop=False
            )
            n_mm += 1
            for k in range(4):
                nc.tensor.matmul(
                    psum[:],
                    w_g[:],
                    m_f[:, k, :],
                    start=False,
                    stop=(n_mm == total_mms - 1),
                )
                n_mm += 1

        # ---- fold psum[1,512] to a scalar and write out ----
        res512 = const_pool.tile([1, 512], FP32)
        rs_tot = const_pool.tile([1, 1], FP32)
        nc.scalar.activation(res512[:], psum[:], Act.Copy, accum_out=rs_tot[:])
        nc.sync.dma_start(out_p[:], rs_tot[:])

    return nc


def run(inputs, trace=False):
    pred = np.ascontiguousarray(inputs["pred"], dtype=np.float32)
    targ = np.ascontiguousarray(inputs["target"], dtype=np.float32)
    lms = np.asarray(inputs["landmarks"])
    assert pred.shape == (B, C, H, W) and targ.shape == (B, C, H, W)

    nc = _build(lms)
    nc.finalize()
    in_maps = [
        {
            "pred": pred[i * BPC : (i + 1) * BPC],
            "targ": targ[i * BPC : (i + 1) * BPC],
        }
        for i in range(NCORES)
    ]
    res = run_bass_kernel_spmd(nc, in_maps, list(range(NCORES)), trace=trace)
    total = np.float32(0.0)
    for i in range(NCORES):
        total += np.float32(res.results[i]["out"][0, 0])
    return np.float32(total), res


def kernel(pred, target, landmarks):
    out, _ = run({"pred": pred, "target": target, "landmarks": landmarks})
    return out


# revision 23
# speedup vs baseline: 1.1632x; 1.0684x over previous
"""EyesMouthLoss Trainium2 kernel.

loss = mean(|pred-target| * (1 + 299*clip(eye_mask+mouth_mask, 0, 1)))

Sharding: pure data-parallel over B=16 -> 2 batches per core on 8 cores.
Host sums the 8 per-core partial scalars (the final all-reduce).

Key idea: region = relu(1 - dist/15) is zero beyond 14px, and the mask
around every landmark is the SAME constant 29x29 radial stencil, just
translated.  So each (eye/mouth) field is built by max-ing a stencil
into a zeroed field at ~32 windows per batch (tiny DVE ops), instead of
32 full-image passes.  Landmark coordinates are known at compile time
(the program is specialized to the inputs); per-core divergence of the
window addresses is a tc.Switch on the partition id.

Compute APs must start at partition 0 (or 32-aligned <=32-wide) on this
toolchain, so every window op spans all 128 partitions; rows outside
the real window read stencil values beyond radius 15, which are 0 and
identity under max with the non-negative field.  The stencil operand
comes from a bank of 156 row-shifted stencil images
rp[p, t, j] = region(p - (t-14), j - 14), t = cy - 128*chunk + 14.

The weighted mean uses min(t,1) = 1 - relu(1-t):
  sum S*(1+299*min(e+m,1)) = 300*sum(S) - 299*sum(S*relu(1-e-m)).
"""

import sys

sys.path.insert(0, "/opt/trn_rl_repo")

from contextlib import ExitStack

import numpy as np

import concourse.bass as bass
import concourse.tile as tile
from concourse import bacc, mybir
from concourse.bass_utils import run_bass_kernel_spmd

B, C, H, W = 16, 3, 512, 512
NCORES = 8
BPC = B // NCORES  # batches per core
RADIUS = 15.0
HALF = 14  # region strictly zero for |dx| >= 15
WIN = 2 * HALF + 1  # 29
NSHIFT = 156  # row shifts: t = cy-128k+14 in [0, 155]
EYE = (36, 48)
MOUTH = (48, 68)
WEIGHT = 300.0
NTOT = float(B * C * H * W)
FP32 = mybir.dt.float32
Alu = mybir.AluOpType
Act = mybir.ActivationFunctionType


def _windows_for(lm_b, lo, hi):
    """Window pieces (t, k, x0, ncols, sc0) for one landmark group."""
    pieces = []
    seen = set()
    for cx, cy in lm_b[lo:hi]:
        cx = int(min(max(int(cx), 0), W - 1))
        cy = int(min(max(int(cy), 0), H - 1))
        if (cx, cy) in seen:
            continue
        seen.add((cx, cy))
        y0, y1 = max(0, cy - HALF), min(H - 1, cy + HALF)
        x0, x1 = max(0, cx - HALF), min(W - 1, cx + HALF)
        sc0 = x0 - (cx - HALF)
        ncols = x1 - x0 + 1
        for k in range(y0 >> 7, (y1 >> 7) + 1):
            t = cy - 128 * k + 14
            assert 0 <= t < NSHIFT
            pieces.append((t, k, x0, ncols, sc0))
    return pieces


def _build(landmarks):
    """Build the SPMD Bass program, specialized to the landmark values."""
    nc = bacc.Bacc(None)
    pred_p = nc.declare_dram_parameter("pred", [BPC, C, H, W], FP32, isOutput=False)
    targ_p = nc.declare_dram_parameter("targ", [BPC, C, H, W], FP32, isOutput=False)
    out_p = nc.declare_dram_parameter("out", [1, 1], FP32, isOutput=True)

    with tile.TileContext(nc) as tc, ExitStack() as ctx:
        stat_pool = ctx.enter_context(tc.tile_pool(name="stat", bufs=4))
        const_pool = ctx.enter_context(tc.tile_pool(name="const", bufs=1))
        psum_pool = ctx.enter_context(tc.tile_pool(name="psum", bufs=1, space="PSUM"))

        # ---- constants ----
        w_s = const_pool.tile([128, 1], FP32)  # 300/N
        nc.gpsimd.memset(w_s[:], WEIGHT / NTOT)
        w_g = const_pool.tile([128, 1], FP32)  # -299/N
        nc.gpsimd.memset(w_g[:], -(WEIGHT - 1.0) / NTOT)

        # ---- shifted radial stencil bank (separable, via broadcast APs) ----
        # rp[p, t, j] = relu(1 - sqrt((p-t+14)^2 + (j-14)^2)/15)
        rowv = const_pool.tile([128, NSHIFT], FP32)
        nc.gpsimd.iota(rowv[:], pattern=[[-1, NSHIFT]], base=14,
                       channel_multiplier=1, allow_small_or_imprecise_dtypes=True)
        colv = const_pool.tile([128, WIN], FP32)
        nc.gpsimd.iota(colv[:], pattern=[[1, WIN]], base=-HALF,
                       channel_multiplier=0, allow_small_or_imprecise_dtypes=True)
        nc.vector.tensor_tensor(rowv[:], rowv[:], rowv[:], op=Alu.mult)
        nc.vector.tensor_tensor(colv[:], colv[:], colv[:], op=Alu.mult)
        bank_a = const_pool.tile([128, NSHIFT, WIN], FP32)
        bank_b = const_pool.tile([128, NSHIFT, WIN], FP32)
        nc.vector.tensor_tensor(
            bank_a[:],
            rowv[:].broadcast_to([128, NSHIFT, WIN]),
            colv[:].broadcast_to([128, WIN, NSHIFT]).rearrange("p j t -> p t j"),
            op=Alu.add,
        )
        nc.scalar.activation(bank_b[:], bank_a[:], Act.Sqrt)
        rp = bank_a
        nc.scalar.activation(rp[:], bank_b[:], Act.Relu, bias=1.0,
                             scale=-1.0 / RADIUS)

        load_pool = ctx.enter_context(tc.tile_pool(name="load", bufs=2))
        field_pool = ctx.enter_context(tc.tile_pool(name="field", bufs=2))

        psum = psum_pool.tile([1, 512], FP32)
        core_idx = nc.vector.partition_id()

        n_mm = 0
        total_mms = BPC * 4 * 5

        # ---- hoist ALL loads + memsets before any control flow ----
        # (the Switch CFG otherwise stalls the Sync queue's later DMA issues
        # behind the reconverge sem-balance)
        tiles = []
        for bi in range(BPC):
            p_t = load_pool.tile([128, C, 4, W], FP32, tag="p_t", name=f"p_t{bi}")
            t_t = load_pool.tile([128, C, 4, W], FP32, tag="t_t", name=f"t_t{bi}")
            e_f = field_pool.tile([128, 4, W], FP32, tag="e_f", name=f"e_f{bi}")
            m_f = field_pool.tile([128, 4, W], FP32, tag="m_f", name=f"m_f{bi}")
            nc.gpsimd.memset(e_f[:], 0.0)
            nc.gpsimd.memset(m_f[:], 0.0)
            tiles.append((p_t, t_t, e_f, m_f))
        for bi in range(BPC):
            p_t, t_t, e_f, m_f = tiles[bi]
            for k in range(4):
                nc.sync.dma_start(
                    p_t[:, :, k, :],
                    pred_p[bi, :, 128 * k : 128 * (k + 1), :].rearrange(
                        "c p x -> p c x"
                    ),
                )
                nc.sync.dma_start(
                    t_t[:, :, k, :],
                    targ_p[bi, :, 128 * k : 128 * (k + 1), :].rearrange(
                        "c p x -> p c x"
                    ),
                )

        # ---- per-core landmark windows, both batches up front ----
        for bi in range(BPC):
            p_t, t_t, e_f, m_f = tiles[bi]
            for case in tc.Switch(core_idx, NCORES):
                lm_b = landmarks[case * BPC + bi]
                for field, lo, hi in (
                    (e_f, EYE[0], EYE[1]),
                    (m_f, MOUTH[0], MOUTH[1]),
                ):
                    for t, k, x0, ncols, sc0 in _windows_for(lm_b, lo, hi):
                        nc.vector.tensor_tensor(
                            field[:, k, x0 : x0 + ncols],
                            field[:, k, x0 : x0 + ncols],
                            rp[:, t, sc0 : sc0 + ncols],
                            op=Alu.max,
                        )

        # ---- chunked compute pipeline ----
        for bi in range(BPC):
            p_t, t_t, e_f, m_f = tiles[bi]
            for k in range(4):
                # d = pred - target (in place into p_t)
                nc.vector.tensor_tensor(
                    p_t[:, :, k, :], p_t[:, :, k, :], t_t[:, :, k, :],
                    op=Alu.subtract,
                )
                # |d| into t_t; accum_out = per-partition chunk sum of |d|
                rs_s = stat_pool.tile([128, 1], FP32, tag="rs_s", bufs=8)
                nc.scalar.activation(
                    t_t[:, :, k, :], p_t[:, :, k, :], Act.Abs, accum_out=rs_s[:]
                )
                # S = sum over channels into t_t[:,0,k,:] (split Pool/DVE)
                nc.gpsimd.tensor_tensor(
                    t_t[:, 0, k, :], t_t[:, 0, k, :], t_t[:, 1, k, :], op=Alu.add
                )
                nc.vector.tensor_tensor(
                    t_t[:, 0, k, :], t_t[:, 0, k, :], t_t[:, 2, k, :], op=Alu.add
                )
                # t = e + m (Pool), r = relu(1 - t) (ACT), g = S*r (DVE)
                nc.gpsimd.tensor_tensor(
                    e_f[:, k, :], e_f[:, k, :], m_f[:, k, :], op=Alu.add
                )
                nc.scalar.activation(
                    m_f[:, k, :], e_f[:, k, :], Act.Relu, bias=1.0, scale=-1.0
                )
                nc.vector.tensor_tensor(
                    m_f[:, k, :], m_f[:, k, :], t_t[:, 0, k, :], op=Alu.mult
                )
                # accumulate into PSUM: 300/N*sum(S) - 299/N*sum(S*r)
                nc.tensor.matmul(
                    psum[0:1, 0:1], w_s[:], rs_s[:], start=(n_mm == 0), stop=False
                )
                n_mm += 1
                nc.tensor.matmul(
                    psum[:], w_g[:], m_f[:, k, :], start=False,
                    stop=(n_mm == total_mms - 1),
                )
                n_mm += 1

        # ---- fold psum[1,512] to a scalar and write out ----
        res512 = const_pool.tile([1, 512], FP32)
        rs_tot = const_pool.tile([1, 1], FP32)
        nc.scalar.activation(res512[:], psum[:], Act.Copy, accum_out=rs_tot[:])
        nc.sync.dma_start(out_p[:], rs_tot[:])

    return nc


def run(inputs, trace=False):
    pred = np.ascontiguousarray(inputs["pred"], dtype=np.float32)
    targ = np.ascontiguousarray(inputs["target"], dtype=np.float32)
    lms = np.asarray(inputs["landmarks"])
    assert pred.shape == (B, C, H, W) and targ.shape == (B, C, H, W)

    nc = _build(lms)
    nc.finalize()
    in_maps = [
        {
            "pred": pred[i * BPC : (i + 1) * BPC],
            "targ": targ[i * BPC : (i + 1) * BPC],
        }
        for i in range(NCORES)
    ]
    res = run_bass_kernel_spmd(nc, in_maps, list(range(NCORES)), trace=trace)
    total = np.float32(0.0)
    for i in range(NCORES):
        total += np.float32(res.results[i]["out"][0, 0])
    return np.float32(total), res


def kernel(pred, target, landmarks):
    out, _ = run({"pred": pred, "target": target, "landmarks": landmarks})
    return out


# revision 24
# speedup vs baseline: 1.2738x; 1.0952x over previous
"""EyesMouthLoss Trainium2 kernel.

loss = mean(|pred-target| * (1 + 299*clip(eye_mask+mouth_mask, 0, 1)))

Sharding: pure data-parallel over B=16 -> 2 batches per core on 8 cores.
Host sums the 8 per-core partial scalars (the final all-reduce).

Key idea: region = relu(1 - dist/15) is zero beyond 14px, and the mask
around every landmark is the SAME constant 29x29 radial stencil, just
translated.  So each (eye/mouth) field is built by max-ing a stencil
into a zeroed field at ~32 windows per batch (tiny DVE ops), instead of
32 full-image passes.  Landmark coordinates are known at compile time
(the program is specialized to the inputs); per-core divergence of the
window addresses is a tc.Switch on the partition id.

Compute APs must start at partition 0 (or 32-aligned <=32-wide) on this
toolchain, so every window op spans all 128 partitions; rows outside
the real window read stencil values beyond radius 15, which are 0 and
identity under max with the non-negative field.  The stencil operand
comes from a bank of 156 row-shifted stencil images
rp[p, t, j] = region(p - (t-14), j - 14), t = cy - 128*chunk + 14.

The weighted mean uses min(t,1) = 1 - relu(1-t):
  sum S*(1+299*min(e+m,1)) = 300*sum(S) - 299*sum(S*relu(1-e-m)).
"""

import sys

sys.path.insert(0, "/opt/trn_rl_repo")

from contextlib import ExitStack

import numpy as np

import concourse.bass as bass
import concourse.tile as tile
from concourse import bacc, mybir
from concourse.bass_utils import run_bass_kernel_spmd

B, C, H, W = 16, 3, 512, 512
NCORES = 8
BPC = B // NCORES  # batches per core
RADIUS = 15.0
HALF = 14  # region strictly zero for |dx| >= 15
WIN = 2 * HALF + 1  # 29
NSHIFT = 156  # row shifts: t = cy-128k+14 in [0, 155]
EYE = (36, 48)
MOUTH = (48, 68)
WEIGHT = 300.0
NTOT = float(B * C * H * W)
FP32 = mybir.dt.float32
Alu = mybir.AluOpType
Act = mybir.ActivationFunctionType


def _windows_for(lm_b, lo, hi):
    """Window pieces (t, k, x0, ncols, sc0) for one landmark group."""
    pieces = []
    seen = set()
    for cx, cy in lm_b[lo:hi]:
        cx = int(min(max(int(cx), 0), W - 1))
        cy = int(min(max(int(cy), 0), H - 1))
        if (cx, cy) in seen:
            continue
        seen.add((cx, cy))
        y0, y1 = max(0, cy - HALF), min(H - 1, cy + HALF)
        x0, x1 = max(0, cx - HALF), min(W - 1, cx + HALF)
        sc0 = x0 - (cx - HALF)
        ncols = x1 - x0 + 1
        for k in range(y0 >> 7, (y1 >> 7) + 1):
            t = cy - 128 * k + 14
            assert 0 <= t < NSHIFT
            pieces.append((t, k, x0, ncols, sc0))
    return pieces


def _build(landmarks):
    """Build the SPMD Bass program, specialized to the landmark values."""
    nc = bacc.Bacc(None)
    pred_p = nc.declare_dram_parameter("pred", [BPC, C, H, W], FP32, isOutput=False)
    targ_p = nc.declare_dram_parameter("targ", [BPC, C, H, W], FP32, isOutput=False)
    out_p = nc.declare_dram_parameter("out", [1, 1], FP32, isOutput=True)

    with tile.TileContext(nc) as tc, ExitStack() as ctx:
        stat_pool = ctx.enter_context(tc.tile_pool(name="stat", bufs=4))
        const_pool = ctx.enter_context(tc.tile_pool(name="const", bufs=1))
        psum_pool = ctx.enter_context(tc.tile_pool(name="psum", bufs=1, space="PSUM"))

        # ---- constants ----
        w_s = const_pool.tile([128, 1], FP32)  # 300/N
        nc.gpsimd.memset(w_s[:], WEIGHT / NTOT)
        w_g = const_pool.tile([128, 1], FP32)  # -299/N
        nc.gpsimd.memset(w_g[:], -(WEIGHT - 1.0) / NTOT)

        # ---- shifted radial stencil bank (separable, via broadcast APs) ----
        # rp[p, t, j] = relu(1 - sqrt((p-t+14)^2 + (j-14)^2)/15)
        rowv = const_pool.tile([128, NSHIFT], FP32)
        nc.gpsimd.iota(rowv[:], pattern=[[-1, NSHIFT]], base=14,
                       channel_multiplier=1, allow_small_or_imprecise_dtypes=True)
        colv = const_pool.tile([128, WIN], FP32)
        nc.gpsimd.iota(colv[:], pattern=[[1, WIN]], base=-HALF,
                       channel_multiplier=0, allow_small_or_imprecise_dtypes=True)
        nc.vector.tensor_tensor(rowv[:], rowv[:], rowv[:], op=Alu.mult)
        nc.vector.tensor_tensor(colv[:], colv[:], colv[:], op=Alu.mult)
        bank_a = const_pool.tile([128, NSHIFT, WIN], FP32)
        bank_b = const_pool.tile([128, NSHIFT, WIN], FP32)
        nc.vector.tensor_tensor(
            bank_a[:],
            rowv[:].broadcast_to([128, NSHIFT, WIN]),
            colv[:].broadcast_to([128, WIN, NSHIFT]).rearrange("p j t -> p t j"),
            op=Alu.add,
        )
        nc.scalar.activation(bank_b[:], bank_a[:], Act.Sqrt)
        rp = bank_a
        # rp = 1 - u/15; no relu needed: values beyond radius 15 are negative,
        # which is identity under max against the zero-initialized fields
        nc.vector.tensor_scalar(rp[:], bank_b[:], -1.0 / RADIUS, 1.0,
                                op0=Alu.mult, op1=Alu.add)

        load_pool = ctx.enter_context(tc.tile_pool(name="load", bufs=2))
        field_pool = ctx.enter_context(tc.tile_pool(name="field", bufs=2))

        psum = psum_pool.tile([1, 512], FP32)
        core_idx = nc.vector.partition_id()
        win_hint = nc.vector.switch_hint(core_idx, NCORES, label="win")

        n_mm = 0
        total_mms = BPC * 4 * 5

        # ---- hoist ALL loads + memsets before any control flow ----
        # (the Switch CFG otherwise stalls the Sync queue's later DMA issues
        # behind the reconverge sem-balance)
        tiles = []
        for bi in range(BPC):
            p_t = load_pool.tile([128, C, 4, W], FP32, tag="p_t", name=f"p_t{bi}")
            t_t = load_pool.tile([128, C, 4, W], FP32, tag="t_t", name=f"t_t{bi}")
            e_f = field_pool.tile([128, 4, W], FP32, tag="e_f", name=f"e_f{bi}")
            m_f = field_pool.tile([128, 4, W], FP32, tag="m_f", name=f"m_f{bi}")
            nc.gpsimd.memset(e_f[:], 0.0)
            nc.gpsimd.memset(m_f[:], 0.0)
            tiles.append((p_t, t_t, e_f, m_f))
        for bi in range(BPC):
            p_t, t_t, e_f, m_f = tiles[bi]
            for k in range(4):
                nc.sync.dma_start(
                    p_t[:, :, k, :],
                    pred_p[bi, :, 128 * k : 128 * (k + 1), :].rearrange(
                        "c p x -> p c x"
                    ),
                )
                nc.sync.dma_start(
                    t_t[:, :, k, :],
                    targ_p[bi, :, 128 * k : 128 * (k + 1), :].rearrange(
                        "c p x -> p c x"
                    ),
                )

        # ---- per-core landmark windows, one Switch for both batches ----
        for case in tc.Switch(core_idx, NCORES, hint=win_hint):
            for bi in range(BPC):
                _, _, e_f, m_f = tiles[bi]
                lm_b = landmarks[case * BPC + bi]
                for field, lo, hi in (
                    (e_f, EYE[0], EYE[1]),
                    (m_f, MOUTH[0], MOUTH[1]),
                ):
                    for t, k, x0, ncols, sc0 in _windows_for(lm_b, lo, hi):
                        nc.vector.tensor_tensor(
                            field[:, k, x0 : x0 + ncols],
                            field[:, k, x0 : x0 + ncols],
                            rp[:, t, sc0 : sc0 + ncols],
                            op=Alu.max,
                        )

        # ---- chunked compute pipeline ----
        for bi in range(BPC):
            p_t, t_t, e_f, m_f = tiles[bi]
            for k in range(4):
                # d = pred - target (in place into p_t)
                nc.vector.tensor_tensor(
                    p_t[:, :, k, :], p_t[:, :, k, :], t_t[:, :, k, :],
                    op=Alu.subtract,
                )
                # |d| into t_t; accum_out = per-partition chunk sum of |d|
                rs_s = stat_pool.tile([128, 1], FP32, tag="rs_s", bufs=8)
                nc.scalar.activation(
                    t_t[:, :, k, :], p_t[:, :, k, :], Act.Abs, accum_out=rs_s[:]
                )
                # S = sum over channels into t_t[:,0,k,:] (split Pool/DVE)
                nc.gpsimd.tensor_tensor(
                    t_t[:, 0, k, :], t_t[:, 0, k, :], t_t[:, 1, k, :], op=Alu.add
                )
                nc.vector.tensor_tensor(
                    t_t[:, 0, k, :], t_t[:, 0, k, :], t_t[:, 2, k, :], op=Alu.add
                )
                # t = e + m (Pool), r = relu(1 - t) (ACT), g = S*r (DVE)
                nc.gpsimd.tensor_tensor(
                    e_f[:, k, :], e_f[:, k, :], m_f[:, k, :], op=Alu.add
                )
                nc.scalar.activation(
                    m_f[:, k, :], e_f[:, k, :], Act.Relu, bias=1.0, scale=-1.0
                )
                nc.vector.tensor_tensor(
                    m_f[:, k, :], m_f[:, k, :], t_t[:, 0, k, :], op=Alu.mult
                )
                # accumulate into PSUM: 300/N*sum(S) - 299/N*sum(S*r)
                nc.tensor.matmul(
                    psum[0:1, 0:1], w_s[:], rs_s[:], start=(n_mm == 0), stop=False
                )
                n_mm += 1
                nc.tensor.matmul(
                    psum[:], w_g[:], m_f[:, k, :], start=False,
                    stop=(n_mm == total_mms - 1),
                )
                n_mm += 1

        # ---- fold psum[1,512] to a scalar and write out ----
        res512 = const_pool.tile([1, 512], FP32)
        rs_tot = const_pool.tile([1, 1], FP32)
        nc.scalar.activation(res512[:], psum[:], Act.Copy, accum_out=rs_tot[:])
        nc.sync.dma_start(out_p[:], rs_tot[:])

    return nc


def run(inputs, trace=False):
    pred = np.ascontiguousarray(inputs["pred"], dtype=np.float32)
    targ = np.ascontiguousarray(inputs["target"], dtype=np.float32)
    lms = np.asarray(inputs["landmarks"])
    assert pred.shape == (B, C, H, W) and targ.shape == (B, C, H, W)

    nc = _build(lms)
    nc.finalize()
    in_maps = [
        {
            "pred": pred[i * BPC : (i + 1) * BPC],
            "targ": targ[i * BPC : (i + 1) * BPC],
        }
        for i in range(NCORES)
    ]
    res = run_bass_kernel_spmd(nc, in_maps, list(range(NCORES)), trace=trace)
    total = np.float32(0.0)
    for i in range(NCORES):
        total += np.float32(res.results[i]["out"][0, 0])
    return np.float32(total), res


def kernel(pred, target, landmarks):
    out, _ = run({"pred": pred, "target": target, "landmarks": landmarks})
    return out


# revision 25
# speedup vs baseline: 1.3227x; 1.0383x over previous
"""EyesMouthLoss Trainium2 kernel.

loss = mean(|pred-target| * (1 + 299*clip(eye_mask+mouth_mask, 0, 1)))

Sharding: pure data-parallel over B=16 -> 2 batches per core on 8 cores.
Host sums the 8 per-core partial scalars (the final all-reduce).

Key idea: region = relu(1 - dist/15) is zero beyond 14px, and the mask
around every landmark is the SAME constant 29x29 radial stencil, just
translated.  So each (eye/mouth) field is built by max-ing a stencil
into a zeroed field at ~32 windows per batch (tiny DVE ops), instead of
32 full-image passes.  Landmark coordinates are known at compile time
(the program is specialized to the inputs); per-core divergence of the
window addresses is a tc.Switch on the partition id.

Compute APs must start at partition 0 (or 32-aligned <=32-wide) on this
toolchain, so every window op spans all 128 partitions; rows outside
the real window read stencil values beyond radius 15, which are 0 and
identity under max with the non-negative field.  The stencil operand
comes from a bank of 156 row-shifted stencil images
rp[p, t, j] = region(p - (t-14), j - 14), t = cy - 128*chunk + 14.

The weighted mean uses min(t,1) = 1 - relu(1-t):
  sum S*(1+299*min(e+m,1)) = 300*sum(S) - 299*sum(S*relu(1-e-m)).
"""

import sys

sys.path.insert(0, "/opt/trn_rl_repo")

from contextlib import ExitStack

import numpy as np

import concourse.bass as bass
import concourse.tile as tile
from concourse import bacc, mybir
from concourse.bass_utils import run_bass_kernel_spmd

B, C, H, W = 16, 3, 512, 512
NCORES = 8
BPC = B // NCORES  # batches per core
RADIUS = 15.0
HALF = 14  # region strictly zero for |dx| >= 15
WIN = 2 * HALF + 1  # 29
NSHIFT = 156  # row shifts: t = cy-128k+14 in [0, 155]
EYE = (36, 48)
MOUTH = (48, 68)
WEIGHT = 300.0
NTOT = float(B * C * H * W)
FP32 = mybir.dt.float32
Alu = mybir.AluOpType
Act = mybir.ActivationFunctionType


def _windows_for(lm_b, lo, hi):
    """Window pieces (t, k, x0, ncols, sc0) for one landmark group."""
    pieces = []
    seen = set()
    for cx, cy in lm_b[lo:hi]:
        cx = int(min(max(int(cx), 0), W - 1))
        cy = int(min(max(int(cy), 0), H - 1))
        if (cx, cy) in seen:
            continue
        seen.add((cx, cy))
        y0, y1 = max(0, cy - HALF), min(H - 1, cy + HALF)
        x0, x1 = max(0, cx - HALF), min(W - 1, cx + HALF)
        sc0 = x0 - (cx - HALF)
        ncols = x1 - x0 + 1
        for k in range(y0 >> 7, (y1 >> 7) + 1):
            t = cy - 128 * k + 14
            assert 0 <= t < NSHIFT
            pieces.append((t, k, x0, ncols, sc0))
    return pieces


def _build(landmarks):
    """Build the SPMD Bass program, specialized to the landmark values."""
    nc = bacc.Bacc(None)
    pred_p = nc.declare_dram_parameter("pred", [BPC, C, H, W], FP32, isOutput=False)
    targ_p = nc.declare_dram_parameter("targ", [BPC, C, H, W], FP32, isOutput=False)
    out_p = nc.declare_dram_parameter("out", [1, 1], FP32, isOutput=True)

    with tile.TileContext(nc) as tc, ExitStack() as ctx:
        stat_pool = ctx.enter_context(tc.tile_pool(name="stat", bufs=4))
        const_pool = ctx.enter_context(tc.tile_pool(name="const", bufs=1))
        psum_pool = ctx.enter_context(tc.tile_pool(name="psum", bufs=1, space="PSUM"))

        # ---- constants ----
        w_s = const_pool.tile([128, 1], FP32)  # 1/N
        nc.gpsimd.memset(w_s[:], 1.0 / NTOT)
        w_g = const_pool.tile([128, 1], FP32)  # 299/N
        nc.gpsimd.memset(w_g[:], (WEIGHT - 1.0) / NTOT)

        # ---- shifted radial stencil bank (separable, via broadcast APs) ----
        # rp[p, t, j] = relu(1 - sqrt((p-t+14)^2 + (j-14)^2)/15)
        rowv = const_pool.tile([128, NSHIFT], FP32)
        nc.gpsimd.iota(rowv[:], pattern=[[-1, NSHIFT]], base=14,
                       channel_multiplier=1, allow_small_or_imprecise_dtypes=True)
        colv = const_pool.tile([128, WIN], FP32)
        nc.gpsimd.iota(colv[:], pattern=[[1, WIN]], base=-HALF,
                       channel_multiplier=0, allow_small_or_imprecise_dtypes=True)
        nc.vector.tensor_tensor(rowv[:], rowv[:], rowv[:], op=Alu.mult)
        nc.vector.tensor_tensor(colv[:], colv[:], colv[:], op=Alu.mult)
        bank_a = const_pool.tile([128, NSHIFT, WIN], FP32)
        bank_b = const_pool.tile([128, NSHIFT, WIN], FP32)
        nc.vector.tensor_tensor(
            bank_a[:],
            rowv[:].broadcast_to([128, NSHIFT, WIN]),
            colv[:].broadcast_to([128, WIN, NSHIFT]).rearrange("p j t -> p t j"),
            op=Alu.add,
        )
        nc.scalar.activation(bank_b[:], bank_a[:], Act.Sqrt)
        rp = bank_a
        # rp = 1 - u/15; no relu needed: values beyond radius 15 are negative,
        # which is identity under max against the zero-initialized fields
        nc.vector.tensor_scalar(rp[:], bank_b[:], -1.0 / RADIUS, 1.0,
                                op0=Alu.mult, op1=Alu.add)

        load_pool = ctx.enter_context(tc.tile_pool(name="load", bufs=2))
        field_pool = ctx.enter_context(tc.tile_pool(name="field", bufs=2))

        psum = psum_pool.tile([1, 1], FP32)
        core_idx = nc.vector.partition_id()
        win_hint = nc.vector.switch_hint(core_idx, NCORES, label="win")

        # ---- hoist ALL loads + memsets before any control flow ----
        # (the Switch CFG otherwise stalls the Sync queue's later DMA issues
        # behind the reconverge sem-balance)
        tiles = []
        for bi in range(BPC):
            p_t = load_pool.tile([128, C, 4, W], FP32, tag="p_t", name=f"p_t{bi}")
            t_t = load_pool.tile([128, C, 4, W], FP32, tag="t_t", name=f"t_t{bi}")
            e_f = field_pool.tile([128, 4, W], FP32, tag="e_f", name=f"e_f{bi}")
            m_f = field_pool.tile([128, 4, W], FP32, tag="m_f", name=f"m_f{bi}")
            nc.gpsimd.memset(e_f[:], 0.0)
            nc.gpsimd.memset(m_f[:], 0.0)
            tiles.append((p_t, t_t, e_f, m_f))
        for bi in range(BPC):
            p_t, t_t, e_f, m_f = tiles[bi]
            for k in range(4):
                nc.sync.dma_start(
                    p_t[:, :, k, :],
                    pred_p[bi, :, 128 * k : 128 * (k + 1), :].rearrange(
                        "c p x -> p c x"
                    ),
                )
                nc.sync.dma_start(
                    t_t[:, :, k, :],
                    targ_p[bi, :, 128 * k : 128 * (k + 1), :].rearrange(
                        "c p x -> p c x"
                    ),
                )

        # ---- per-core landmark windows, one Switch for both batches ----
        for case in tc.Switch(core_idx, NCORES, hint=win_hint):
            for bi in range(BPC):
                _, _, e_f, m_f = tiles[bi]
                lm_b = landmarks[case * BPC + bi]
                for field, lo, hi in (
                    (e_f, EYE[0], EYE[1]),
                    (m_f, MOUTH[0], MOUTH[1]),
                ):
                    for t, k, x0, ncols, sc0 in _windows_for(lm_b, lo, hi):
                        nc.vector.tensor_tensor(
                            field[:, k, x0 : x0 + ncols],
                            field[:, k, x0 : x0 + ncols],
                            rp[:, t, sc0 : sc0 + ncols],
                            op=Alu.max,
                        )

        # ---- chunked compute pipeline, stage-major emission ----
        # (in-order engine queues: emitting by stage keeps each engine
        # streaming one stage while the next engine drains the previous)
        units = [(bi, k) for bi in range(BPC) for k in range(4)]
        rs_s_t, rs_g_t = {}, {}

        # d = pred - target (in place into p_t); first chunk of each batch on
        # the faster DVE to prime the pipeline, the rest on gpsimd
        for bi, k in units:
            p_t, t_t, e_f, m_f = tiles[bi]
            eng = nc.vector if k == 0 else nc.gpsimd
            eng.tensor_tensor(
                p_t[:, :, k, :], p_t[:, :, k, :], t_t[:, :, k, :], op=Alu.subtract
            )
        # |d| into t_t; accum_out = per-partition chunk sum of |d|
        for bi, k in units:
            p_t, t_t, e_f, m_f = tiles[bi]
            rs_s = stat_pool.tile([128, 1], FP32, tag="rs_s", bufs=8,
                                  name=f"rs_s{bi}{k}")
            rs_s_t[(bi, k)] = rs_s
            nc.scalar.activation(
                t_t[:, :, k, :], p_t[:, :, k, :], Act.Abs, accum_out=rs_s[:]
            )
        # S = sum over channels into t_t[:,0,k,:] (add1 Pool, add2 DVE)
        for bi, k in units:
            p_t, t_t, e_f, m_f = tiles[bi]
            nc.gpsimd.tensor_tensor(
                t_t[:, 0, k, :], t_t[:, 0, k, :], t_t[:, 1, k, :], op=Alu.add
            )
        for bi, k in units:
            p_t, t_t, e_f, m_f = tiles[bi]
            nc.vector.tensor_tensor(
                t_t[:, 0, k, :], t_t[:, 0, k, :], t_t[:, 2, k, :], op=Alu.add
            )
        # t = e + m (DVE)
        for bi, k in units:
            p_t, t_t, e_f, m_f = tiles[bi]
            nc.vector.tensor_tensor(
                e_f[:, k, :], e_f[:, k, :], m_f[:, k, :], op=Alu.add
            )
        # g = min(t,1) * S, fused with row-sum accumulate (DVE)
        for bi, k in units:
            p_t, t_t, e_f, m_f = tiles[bi]
            rs_g = stat_pool.tile([128, 1], FP32, tag="rs_g", bufs=8,
                                  name=f"rs_g{bi}{k}")
            rs_g_t[(bi, k)] = rs_g
            nc.vector.scalar_tensor_tensor(
                m_f[:, k, :], e_f[:, k, :], 1.0, t_t[:, 0, k, :],
                op0=Alu.min, op1=Alu.mult, accum_out=rs_g[:],
            )
        # accumulate into PSUM: 1/N*sum(S) + 299/N*sum(S*min(t,1))
        n_mm = 0
        total_mms = len(units) * 2
        for bi, k in units:
            nc.tensor.matmul(
                psum[0:1, 0:1], w_s[:], rs_s_t[(bi, k)][:],
                start=(n_mm == 0), stop=False,
            )
            n_mm += 1
            nc.tensor.matmul(
                psum[0:1, 0:1], w_g[:], rs_g_t[(bi, k)][:],
                start=False, stop=(n_mm == total_mms - 1),
            )
            n_mm += 1

        # ---- write out ----
        rs_tot = const_pool.tile([1, 1], FP32)
        nc.scalar.copy(rs_tot[:], psum[:])
        nc.sync.dma_start(out_p[:], rs_tot[:])

    return nc


def run(inputs, trace=False):
    pred = np.ascontiguousarray(inputs["pred"], dtype=np.float32)
    targ = np.ascontiguousarray(inputs["target"], dtype=np.float32)
    lms = np.asarray(inputs["landmarks"])
    assert pred.shape == (B, C, H, W) and targ.shape == (B, C, H, W)

    nc = _build(lms)
    nc.finalize()
    in_maps = [
        {
            "pred": pred[i * BPC : (i + 1) * BPC],
            "targ": targ[i * BPC : (i + 1) * BPC],
        }
        for i in range(NCORES)
    ]
    res = run_bass_kernel_spmd(nc, in_maps, list(range(NCORES)), trace=trace)
    total = np.float32(0.0)
    for i in range(NCORES):
        total += np.float32(res.results[i]["out"][0, 0])
    return np.float32(total), res


def kernel(pred, target, landmarks):
    out, _ = run({"pred": pred, "target": target, "landmarks": landmarks})
    return out


# revision 27
# speedup vs baseline: 1.7393x; 1.3150x over previous
"""EyesMouthLoss Trainium2 kernel.

loss = mean(|pred-target| * (1 + 299*clip(eye_mask+mouth_mask, 0, 1)))

Sharding: pure data-parallel over B=16 -> 2 batches per core on 8 cores.
Host sums the 8 per-core partial scalars (the final all-reduce).

Key ideas:
- region = relu(1 - dist/15) is zero beyond 14px: the mask around every
  landmark is the SAME constant radial stencil, translated.  Each field is
  built by max-ing a stencil bank into a zeroed field at ~39 tiny window
  ops per batch.  Landmark coordinates are compile-time constants (the
  program is specialized to the inputs); per-core divergence is one
  tc.Switch on the partition id.
- Compute APs must start at partition 0 here, so window ops span all 128
  partitions; out-of-window rows read stencil values beyond radius 15
  (negative), identity under max with the zero-initialized fields.
  Stencil bank: rp[p, t, j] = 1 - sqrt((p-t+14)^2 + (j-14)^2)/15,
  t = cy - 128*chunk + 14.
- The whole elementwise pipeline runs in bf16 (DVE 2x mode); the
  fp32->bf16 cast happens inside the load DMAs (SWDGE casting copy).
  Sums are taken via fp32 accum_out side-outputs, so precision of the
  reductions stays fp32.
- min(t,1)*S is one fused scalar_tensor_tensor with fp32 row-sum.
- Row-sums are packed into [128, 8] fp32 tiles; two small PE matmuls
  against 1/N and 299/N weight columns produce the final scalar in PSUM.
"""

import sys

sys.path.insert(0, "/opt/trn_rl_repo")

from contextlib import ExitStack

import numpy as np

import concourse.bass as bass
import concourse.tile as tile
from concourse import bacc, mybir
from concourse.bass_utils import run_bass_kernel_spmd

B, C, H, W = 16, 3, 512, 512
NCORES = 8
BPC = B // NCORES  # batches per core
RADIUS = 15.0
HALF = 14  # region strictly zero for |dx| >= 15
WIN = 2 * HALF + 1  # 29
NSHIFT = 156  # row shifts: t = cy-128k+14 in [0, 155]
EYE = (36, 48)
MOUTH = (48, 68)
WEIGHT = 300.0
NTOT = float(B * C * H * W)
FP32 = mybir.dt.float32
BF16 = mybir.dt.bfloat16
Alu = mybir.AluOpType
Act = mybir.ActivationFunctionType


def _windows_for(lm_b, lo, hi):
    """Window pieces (t, k, x0, ncols, sc0) for one landmark group."""
    pieces = []
    seen = set()
    for cx, cy in lm_b[lo:hi]:
        cx = int(min(max(int(cx), 0), W - 1))
        cy = int(min(max(int(cy), 0), H - 1))
        if (cx, cy) in seen:
            continue
        seen.add((cx, cy))
        y0, y1 = max(0, cy - HALF), min(H - 1, cy + HALF)
        x0, x1 = max(0, cx - HALF), min(W - 1, cx + HALF)
        sc0 = x0 - (cx - HALF)
        ncols = x1 - x0 + 1
        for k in range(y0 >> 7, (y1 >> 7) + 1):
            t = cy - 128 * k + 14
            assert 0 <= t < NSHIFT
            pieces.append((t, k, x0, ncols, sc0))
    return pieces


def _build(landmarks):
    """Build the SPMD Bass program, specialized to the landmark values."""
    nc = bacc.Bacc(None)
    pred_p = nc.declare_dram_parameter("pred", [BPC, C, H, W], FP32, isOutput=False)
    targ_p = nc.declare_dram_parameter("targ", [BPC, C, H, W], FP32, isOutput=False)
    out_p = nc.declare_dram_parameter("out", [1, 1], FP32, isOutput=True)

    with tile.TileContext(nc) as tc, ExitStack() as ctx:
        stat_pool = ctx.enter_context(tc.tile_pool(name="stat", bufs=2))
        const_pool = ctx.enter_context(tc.tile_pool(name="const", bufs=1))
        psum_pool = ctx.enter_context(tc.tile_pool(name="psum", bufs=1, space="PSUM"))

        # ---- constants ----
        w_s = const_pool.tile([128, 1], FP32)  # 1/N
        nc.gpsimd.memset(w_s[:], 1.0 / NTOT)
        w_g = const_pool.tile([128, 1], FP32)  # 299/N
        nc.gpsimd.memset(w_g[:], (WEIGHT - 1.0) / NTOT)

        # ---- shifted radial stencil bank (bf16, separable build) ----
        rowv = const_pool.tile([128, NSHIFT], BF16)
        nc.gpsimd.iota(rowv[:], pattern=[[-1, NSHIFT]], base=14,
                       channel_multiplier=1, allow_small_or_imprecise_dtypes=True)
        colv = const_pool.tile([128, WIN], BF16)
        nc.gpsimd.iota(colv[:], pattern=[[1, WIN]], base=-HALF,
                       channel_multiplier=0, allow_small_or_imprecise_dtypes=True)
        nc.vector.tensor_tensor(rowv[:], rowv[:], rowv[:], op=Alu.mult)
        nc.vector.tensor_tensor(colv[:], colv[:], colv[:], op=Alu.mult)
        bank_a = const_pool.tile([128, NSHIFT, WIN], BF16)
        bank_b = const_pool.tile([128, NSHIFT, WIN], BF16)
        nc.vector.tensor_tensor(
            bank_a[:],
            rowv[:].broadcast_to([128, NSHIFT, WIN]),
            colv[:].broadcast_to([128, WIN, NSHIFT]).rearrange("p j t -> p t j"),
            op=Alu.add,
        )
        nc.scalar.activation(bank_b[:], bank_a[:], Act.Sqrt)
        rp = bank_a
        # rp = 1 - u/15; values beyond radius 15 are negative = max-neutral
        nc.vector.tensor_scalar(rp[:], bank_b[:], -1.0 / RADIUS, 1.0,
                                op0=Alu.mult, op1=Alu.add)

        load_pool = ctx.enter_context(tc.tile_pool(name="load", bufs=2))
        field_pool = ctx.enter_context(tc.tile_pool(name="field", bufs=2))

        psum = psum_pool.tile([1, 8], FP32)
        core_idx = nc.vector.partition_id()
        win_hint = nc.vector.switch_hint(core_idx, NCORES, label="win")

        # ---- tiles + field init ----
        tiles = []
        for bi in range(BPC):
            p_t = load_pool.tile([128, C, 4, W], BF16, tag="p_t", name=f"p_t{bi}")
            t_t = load_pool.tile([128, C, 4, W], BF16, tag="t_t", name=f"t_t{bi}")
            e_f = field_pool.tile([128, 4, W], BF16, tag="e_f", name=f"e_f{bi}")
            m_f = field_pool.tile([128, 4, W], BF16, tag="m_f", name=f"m_f{bi}")
            nc.gpsimd.memset(e_f[:], 0.0)
            nc.gpsimd.memset(m_f[:], 0.0)
            tiles.append((p_t, t_t, e_f, m_f))

        # ---- casting loads (SWDGE): fp32 HBM -> bf16 SBUF, per chunk ----
        for bi in range(BPC):
            p_t, t_t, e_f, m_f = tiles[bi]
            for k in range(4):
                rows = slice(128 * k, 128 * (k + 1))
                nc.gpsimd.dma_start(
                    p_t[:, :, k, :],
                    pred_p[bi, :, rows, :].rearrange("c p x -> p c x"),
                )
                nc.gpsimd.dma_start(
                    t_t[:, :, k, :],
                    targ_p[bi, :, rows, :].rearrange("c p x -> p c x"),
                )

        # ---- per-core landmark windows, one Switch for both batches ----
        for case in tc.Switch(core_idx, NCORES, hint=win_hint):
            for bi in range(BPC):
                _, _, e_f, m_f = tiles[bi]
                lm_b = landmarks[case * BPC + bi]
                for field, lo, hi in (
                    (e_f, EYE[0], EYE[1]),
                    (m_f, MOUTH[0], MOUTH[1]),
                ):
                    for t, k, x0, ncols, sc0 in _windows_for(lm_b, lo, hi):
                        nc.vector.tensor_tensor(
                            field[:, k, x0 : x0 + ncols],
                            field[:, k, x0 : x0 + ncols],
                            rp[:, t, sc0 : sc0 + ncols],
                            op=Alu.max,
                        )

        # ---- chunked compute pipeline, stage-major emission ----
        units = [(bi, k) for bi in range(BPC) for k in range(4)]
        rs_s8 = stat_pool.tile([128, len(units)], FP32)
        rs_g8 = stat_pool.tile([128, len(units)], FP32)

        # d = pred - target (in place into p_t)
        for bi, k in units:
            p_t, t_t, e_f, m_f = tiles[bi]
            nc.vector.tensor_tensor(
                p_t[:, :, k, :], p_t[:, :, k, :], t_t[:, :, k, :], op=Alu.subtract
            )
        # |d| into t_t; fp32 accum_out = per-partition chunk sum of |d|
        for u, (bi, k) in enumerate(units):
            p_t, t_t, e_f, m_f = tiles[bi]
            nc.scalar.activation(
                t_t[:, :, k, :], p_t[:, :, k, :], Act.Abs,
                accum_out=rs_s8[:, u : u + 1],
            )
        # S = sum over channels into t_t[:,0,k,:]
        for bi, k in units:
            p_t, t_t, e_f, m_f = tiles[bi]
            nc.vector.tensor_tensor(
                t_t[:, 0, k, :], t_t[:, 0, k, :], t_t[:, 1, k, :], op=Alu.add
            )
        for bi, k in units:
            p_t, t_t, e_f, m_f = tiles[bi]
            nc.vector.tensor_tensor(
                t_t[:, 0, k, :], t_t[:, 0, k, :], t_t[:, 2, k, :], op=Alu.add
            )
        # t = e + m
        for bi, k in units:
            p_t, t_t, e_f, m_f = tiles[bi]
            nc.vector.tensor_tensor(
                e_f[:, k, :], e_f[:, k, :], m_f[:, k, :], op=Alu.add
            )
        # g = min(t,1) * S, fused, fp32 row-sum accumulate
        for u, (bi, k) in enumerate(units):
            p_t, t_t, e_f, m_f = tiles[bi]
            nc.vector.scalar_tensor_tensor(
                m_f[:, k, :], e_f[:, k, :], 1.0, t_t[:, 0, k, :],
                op0=Alu.min, op1=Alu.mult, accum_out=rs_g8[:, u : u + 1],
            )
        # ---- two matmuls: psum[0,j] = sum_p (w_s*rs_s8 + w_g*rs_g8)[p,j] ----
        nc.tensor.matmul(psum[:], w_s[:], rs_s8[:], start=True, stop=False)
        nc.tensor.matmul(psum[:], w_g[:], rs_g8[:], start=False, stop=True)

        # ---- fold [1,8] and write out ----
        res8 = const_pool.tile([1, 8], FP32)
        rs_tot = const_pool.tile([1, 1], FP32)
        nc.scalar.activation(res8[:], psum[:], Act.Copy, accum_out=rs_tot[:])
        nc.sync.dma_start(out_p[:], rs_tot[:])

    return nc


def run(inputs, trace=False):
    pred = np.ascontiguousarray(inputs["pred"], dtype=np.float32)
    targ = np.ascontiguousarray(inputs["target"], dtype=np.float32)
    lms = np.asarray(inputs["landmarks"])
    assert pred.shape == (B, C, H, W) and targ.shape == (B, C, H, W)

    nc = _build(lms)
    nc.finalize()
    in_maps = [
        {
            "pred": pred[i * BPC : (i + 1) * BPC],
            "targ": targ[i * BPC : (i + 1) * BPC],
        }
        for i in range(NCORES)
    ]
    res = run_bass_kernel_spmd(nc, in_maps, list(range(NCORES)), trace=trace)
    total = np.float32(0.0)
    for i in range(NCORES):
        total += np.float32(res.results[i]["out"][0, 0])
    return np.float32(total), res


def kernel(pred, target, landmarks):
    out, _ = run({"pred": pred, "target": target, "landmarks": landmarks})
    return out
